# revision 6
# baseline (speedup 1.0000x reference)
"""Bahdanau attention kernel for 8 Trainium2 NeuronCores.

Math note: in the reference,
    score = (tanh(enc @ Wh + bh) + (dec @ Ws + bs)[:, None, :]) @ Wv + bv
    attn  = softmax(score, axis=T)
the decoder projection term and bv are constant across T, and softmax is
shift-invariant along T — so the decoder branch cancels exactly and both
outputs depend only on softmax_T(tanh(enc @ Wh + bh) @ Wv) and enc itself.

Sharding: data-parallel over batch (8 batches per core). Per core:
  - big matmul enclinT[u, bt] = Wh-tile (stationary) x encT[h, t] (moving)
    in bf16, fp32 PSUM accumulation
  - tanh (+bh per-partition bias) on ScalarE -> bf16 tiles
  - score[1, t] = Wv-tile (stationary) x tanh tiles, PSUM-accumulated;
    score matmuls for batch b are issued inside batch b+1's big-MM stream
    so the PE never stalls on the tanh dependency
  - per-batch softmax on [1, 512] (max / exp+accum / reciprocal)
  - context in fp32: attn row broadcast across partitions (GpSimdE), then
    per h-tile multiply (VectorE) + free-dim reduce (VectorE mid-kernel,
    ScalarE accum for the last batch to shorten the tail)
Host pre-computes encT (fp32 + bf16) and a j-tiled Wh per core; outputs are
gathered and context^T is rearranged on host.
"""

from contextlib import ExitStack

import numpy as np
import ml_dtypes

import concourse.bass as bass
import concourse.tile as tile
from concourse import bacc, mybir
from concourse.bass_utils import run_bass_kernel_spmd

B, T, H, U = 64, 512, 1024, 1024
NCORES = 8
BL = B // NCORES          # 8 batches per core
P = 128
NH = H // P               # 8 h-tiles
NU = U // P               # 8 u-tiles

f32 = mybir.dt.float32
bf16 = mybir.dt.bfloat16
AF = mybir.ActivationFunctionType
ALU = mybir.AluOpType
BF16 = ml_dtypes.bfloat16


def build_kernel_body(tc, aps, body_ctx):
    nc = tc.nc
    enc_bf_d = aps["enc_bf"]      # [H, BL*T] bf16
    enc_f32_d = aps["enc_f32"]    # [H, BL*T] f32
    wh_d = aps["wh"]              # [NU, H, P] bf16 (j-tiled)
    wv_d = aps["wv"]              # [P, NU] bf16  (column j = Wv[j*128:(j+1)*128])
    bh_d = aps["bh"]              # [P, NU] f32
    attn_d = aps["attn"]          # [BL, T] f32 out
    ctxt_d = aps["ctxt"]          # [P, NH*BL] f32 out (context^T columns)

    ctx = body_ctx
    cpool = ctx.enter_context(tc.tile_pool(name="const", bufs=1))
    ebf_pool = ctx.enter_context(tc.tile_pool(name="ebf", bufs=2))
    ef_pool = ctx.enter_context(tc.tile_pool(name="ef", bufs=2))
    th_pool = ctx.enter_context(tc.tile_pool(name="th", bufs=2))
    bc_pool = ctx.enter_context(tc.tile_pool(name="bc", bufs=2))
    scr_pool = ctx.enter_context(tc.tile_pool(name="scr", bufs=3))
    row_pool = ctx.enter_context(tc.tile_pool(name="row", bufs=2))
    ctx_pool = ctx.enter_context(tc.tile_pool(name="ctxt", bufs=1))
    pe_pool = ctx.enter_context(tc.tile_pool(name="pe", bufs=4, space="PSUM"))
    sc_pool = ctx.enter_context(tc.tile_pool(name="sc", bufs=2, space="PSUM"))

    # --- constants; j=0 Wh tiles first so the first matmul group can start
    # after ~1.25 MB of DMA instead of ~3 MB.
    wh_t = {}
    for j in [0]:
        for h in range(NH):
            t_ = cpool.tile([P, P], bf16, tag=f"wh{h}_{j}")
            nc.sync.dma_start(t_[:], wh_d[j, h * P:(h + 1) * P, :])
            wh_t[(h, j)] = t_

    # batch-0 enc bf16 tiles next (also required for the first group)
    ebf_cur = []
    for h in range(NH):
        t_ = ebf_pool.tile([P, T], bf16, tag=f"ebf{h}")
        nc.sync.dma_start(t_[:], enc_bf_d[h * P:(h + 1) * P, 0:T])
        ebf_cur.append(t_)

    for j in range(1, NU):
        for h in range(NH):
            t_ = cpool.tile([P, P], bf16, tag=f"wh{h}_{j}")
            nc.sync.dma_start(t_[:], wh_d[j, h * P:(h + 1) * P, :])
            wh_t[(h, j)] = t_
    wv_sb = cpool.tile([P, NU], bf16, tag="wv")
    nc.sync.dma_start(wv_sb[:], wv_d[:, :])
    bh_sb = cpool.tile([P, NU], f32, tag="bh")
    nc.sync.dma_start(bh_sb[:], bh_d[:, :])

    ctx_all = ctx_pool.tile([P, NH * BL], f32, tag="ctxall")

    # fp32 enc tiles stream on the gpsimd queue (sync queue carries bf16+Wh)
    def load_ef(b):
        tiles = []
        for h in range(NH):
            t_ = ef_pool.tile([P, T], f32, tag=f"ef{h}")
            nc.gpsimd.dma_start(t_[:], enc_f32_d[h * P:(h + 1) * P,
                                                 b * T:(b + 1) * T])
            tiles.append(t_)
        return tiles

    ef_cur = load_ef(0)

    state = {}  # batch -> (tanh tiles, ef tiles)

    def issue_score_and_context(b, tanh_ts, ef, last):
        """Score matmuls + softmax + context for batch b."""
        ps = sc_pool.tile([1, T], f32, tag="score")
        for j in range(NU):
            nc.tensor.matmul(
                ps[:], wv_sb[:, j:j + 1], tanh_ts[j][:],
                start=(j == 0), stop=(j == NU - 1), skip_group_check=True,
            )
        nmax = row_pool.tile([1, 1], f32, tag="nmax")
        nc.vector.tensor_reduce(nmax[:], ps[:], axis=mybir.AxisListType.X,
                                op=ALU.max, negate=True)
        erow = row_pool.tile([1, T], f32, tag="erow")
        ssum = row_pool.tile([1, 1], f32, tag="ssum")
        nc.scalar.activation(erow[:], ps[:], AF.Exp, bias=nmax[:],
                             accum_out=ssum[:])
        rrec = row_pool.tile([1, 1], f32, tag="rrec")
        nc.vector.reciprocal(rrec[:], ssum[:])
        arow = row_pool.tile([1, T], f32, tag="arow")
        nc.vector.tensor_scalar_mul(arow[:], erow[:], rrec[:])
        nc.scalar.dma_start(attn_d[b:b + 1, :], arow[:])

        bc = bc_pool.tile([P, T], f32, tag="bc")
        nc.gpsimd.partition_broadcast(bc[:], arow[:])
        for h in range(NH):
            scr = scr_pool.tile([P, T], f32, tag="scr")
            nc.vector.tensor_mul(scr[:], ef[h][:], bc[:])
            col = ctx_all[:, h * BL + b:h * BL + b + 1]
            if last:
                # keep the exposed tail short: reduce on ScalarE via the
                # activation accumulator while VectorE keeps multiplying
                scr2 = scr_pool.tile([P, T], f32, tag="scr2")
                nc.scalar.activation(scr2[:], scr[:], AF.Identity, bias=0.0,
                                     accum_out=col)
            else:
                nc.vector.tensor_reduce(col, scr[:],
                                        axis=mybir.AxisListType.X, op=ALU.add)

    for b in range(BL):
        ebf, ef = ebf_cur, ef_cur
        tanh_ts = []
        for j in range(NU):
            pe = pe_pool.tile([P, T], f32, tag="pe")
            for h in range(NH):
                nc.tensor.matmul(
                    pe[:], wh_t[(h, j)][:], ebf[h][:],
                    start=(h == 0), stop=(h == NH - 1),
                )
            th = th_pool.tile([P, T], bf16, tag=f"th{j}")
            nc.scalar.activation(th[:], pe[:], AF.Tanh, bias=bh_sb[:, j:j + 1])
            tanh_ts.append(th)
            if j == 0 and b > 0:
                # previous batch's score matmuls slot in here: their tanh
                # inputs are long since ready, so the PE stays dense
                pt, pef = state.pop(b - 1)
                issue_score_and_context(b - 1, pt, pef, last=False)
            if j == 1 and b + 1 < BL:
                # prefetch next batch while this one computes
                nxt = []
                for h in range(NH):
                    t_ = ebf_pool.tile([P, T], bf16, tag=f"ebf{h}")
                    nc.sync.dma_start(
                        t_[:], enc_bf_d[h * P:(h + 1) * P,
                                        (b + 1) * T:(b + 2) * T])
                    nxt.append(t_)
                ebf_cur = nxt
                ef_cur = load_ef(b + 1)
        state[b] = (tanh_ts, ef)

    pt, pef = state.pop(BL - 1)
    issue_score_and_context(BL - 1, pt, pef, last=True)

    nc.scalar.dma_start(ctxt_d[:, :], ctx_all[:])


def build_nc():
    nc = bacc.Bacc("TRN2", target_bir_lowering=False, debug=False,
                   num_devices=NCORES)
    aps = {
        "enc_bf": nc.dram_tensor("enc_bf", [H, BL * T], bf16,
                                 kind="ExternalInput").ap(),
        "enc_f32": nc.dram_tensor("enc_f32", [H, BL * T], f32,
                                  kind="ExternalInput").ap(),
        "wh": nc.dram_tensor("wh", [NU, H, P], bf16,
                             kind="ExternalInput").ap(),
        "wv": nc.dram_tensor("wv", [P, NU], bf16, kind="ExternalInput").ap(),
        "bh": nc.dram_tensor("bh", [P, NU], f32, kind="ExternalInput").ap(),
        "attn": nc.dram_tensor("attn", [BL, T], f32,
                               kind="ExternalOutput").ap(),
        "ctxt": nc.dram_tensor("ctxt", [P, NH * BL], f32,
                               kind="ExternalOutput").ap(),
    }
    with tile.TileContext(nc) as tc:
        with ExitStack() as body_ctx:
            build_kernel_body(tc, aps, body_ctx)
    nc.compile()
    return nc


def make_in_maps(enc_output, Wh, bh, Wv):
    enc = np.ascontiguousarray(np.asarray(enc_output, dtype=np.float32))
    wh = np.asarray(Wh, dtype=np.float32)
    # j-tiled Wh: wh_tiled[j, h*P+p, c] = Wh[h*P+p, j*P+c]
    wh_tiled = np.ascontiguousarray(
        wh.reshape(H, NU, P).transpose(1, 0, 2)).astype(BF16)
    wv_t = np.ascontiguousarray(
        np.asarray(Wv, dtype=np.float32).reshape(NU, P).T).astype(BF16)
    bh_t = np.ascontiguousarray(
        np.asarray(bh, dtype=np.float32).reshape(NU, P).T)
    in_maps = []
    for c in range(NCORES):
        shard = enc[c * BL:(c + 1) * BL].reshape(BL * T, H)
        encT = np.ascontiguousarray(shard.T)          # [H, BL*T] f32
        in_maps.append({
            "enc_bf": encT.astype(BF16),
            "enc_f32": encT,
            "wh": wh_tiled,
            "wv": wv_t,
            "bh": bh_t,
        })
    return in_maps


_NC_CACHE = None


def kernel(dec_hidden, enc_output, Wh, bh, Ws, bs, Wv, bv, **_unused):
    global _NC_CACHE
    if _NC_CACHE is None:
        _NC_CACHE = build_nc()
    nc = _NC_CACHE
    in_maps = make_in_maps(enc_output, Wh, bh, Wv)
    res = run_bass_kernel_spmd(nc, in_maps, list(range(NCORES))).results
    attn = np.concatenate([res[c]["attn"] for c in range(NCORES)], axis=0)
    ctx_parts = []
    for c in range(NCORES):
        ct = res[c]["ctxt"]                           # [P, NH*BL]
        ct = ct.reshape(P, NH, BL).transpose(2, 1, 0).reshape(BL, H)
        ctx_parts.append(np.ascontiguousarray(ct))
    context = np.concatenate(ctx_parts, axis=0)
    return context.astype(np.float32), attn.astype(np.float32)


# revision 7
# speedup vs baseline: 1.2454x; 1.2454x over previous
"""Bahdanau attention kernel for 8 Trainium2 NeuronCores.

Math note: in the reference,
    score = (tanh(enc @ Wh + bh) + (dec @ Ws + bs)[:, None, :]) @ Wv + bv
    attn  = softmax(score, axis=T)
the decoder projection term and bv are constant across T, and softmax is
shift-invariant along T — so the decoder branch cancels exactly and both
outputs depend only on softmax_T(tanh(enc @ Wh + bh) @ Wv) and enc itself.

Sharding: data-parallel over batch (8 batches per core). Per core:
  - big matmul enclinT[u, bt] = Wh-tile (stationary) x encT[h, t] (moving)
    in bf16, fp32 PSUM accumulation
  - tanh (+bh per-partition bias) on ScalarE -> bf16 tiles
  - score[1, t] = Wv-tile (stationary) x tanh tiles, PSUM-accumulated;
    score matmuls for batch b are issued inside batch b+1's big-MM stream
    so the PE never stalls on the tanh dependency
  - per-batch softmax on [1, 512] (max / exp+accum / reciprocal)
  - context in fp32: attn row broadcast across partitions (GpSimdE), then
    per h-tile multiply (VectorE) + free-dim reduce (VectorE mid-kernel,
    ScalarE accum for the last batch to shorten the tail)
Host pre-computes encT (fp32 + bf16) and a j-tiled Wh per core; outputs are
gathered and context^T is rearranged on host.
"""

from contextlib import ExitStack

import numpy as np
import ml_dtypes

import concourse.bass as bass
import concourse.tile as tile
from concourse import bacc, mybir
from concourse.bass_utils import run_bass_kernel_spmd

B, T, H, U = 64, 512, 1024, 1024
NCORES = 8
BL = B // NCORES          # 8 batches per core
P = 128
NH = H // P               # 8 h-tiles
NU = U // P               # 8 u-tiles

f32 = mybir.dt.float32
bf16 = mybir.dt.bfloat16
AF = mybir.ActivationFunctionType
ALU = mybir.AluOpType
BF16 = ml_dtypes.bfloat16


def build_kernel_body(tc, aps, body_ctx):
    nc = tc.nc
    enc_bf_d = aps["enc_bf"]      # [H, BL*T] bf16
    enc_f32_d = aps["enc_f32"]    # [H, BL*T] f32
    wh_d = aps["wh"]              # [NU, H, P] bf16 (j-tiled)
    wv_d = aps["wv"]              # [P, NU] bf16  (column j = Wv[j*128:(j+1)*128])
    bh_d = aps["bh"]              # [P, NU] f32
    attn_d = aps["attn"]          # [BL, T] f32 out
    ctxt_d = aps["ctxt"]          # [P, NH*BL] f32 out (context^T columns)

    ctx = body_ctx
    cpool = ctx.enter_context(tc.tile_pool(name="const", bufs=1))
    ebf_pool = ctx.enter_context(tc.tile_pool(name="ebf", bufs=2))
    ef_pool = ctx.enter_context(tc.tile_pool(name="ef", bufs=2))
    th_pool = ctx.enter_context(tc.tile_pool(name="th", bufs=2))
    bc_pool = ctx.enter_context(tc.tile_pool(name="bc", bufs=2))
    scr_pool = ctx.enter_context(tc.tile_pool(name="scr", bufs=3))
    row_pool = ctx.enter_context(tc.tile_pool(name="row", bufs=3))
    ctx_pool = ctx.enter_context(tc.tile_pool(name="ctxt", bufs=1))
    pe_pool = ctx.enter_context(tc.tile_pool(name="pe", bufs=4, space="PSUM"))
    sc_pool = ctx.enter_context(tc.tile_pool(name="sc", bufs=2, space="PSUM"))

    # --- constants; j=0 Wh tile first so the first matmul group can start
    # after ~1.25 MB of DMA instead of ~3 MB.
    def load_wh(j):
        t_ = cpool.tile([P, H], bf16, tag=f"wh{j}")
        nc.sync.dma_start(t_[:].rearrange("p (h c) -> p h c", h=NH),
                          wh_d[j].rearrange("(h p) c -> p h c", p=P))
        return t_

    wh_t = {0: load_wh(0)}

    # batch-0 enc bf16 tiles next (also required for the first group)
    ebf_cur = []
    for h in range(NH):
        t_ = ebf_pool.tile([P, T], bf16, tag=f"ebf{h}")
        nc.sync.dma_start(t_[:], enc_bf_d[h * P:(h + 1) * P, 0:T])
        ebf_cur.append(t_)

    for j in range(1, NU):
        wh_t[j] = load_wh(j)
    wv_sb = cpool.tile([P, NU], bf16, tag="wv")
    nc.sync.dma_start(wv_sb[:], wv_d[:, :])
    bh_sb = cpool.tile([P, NU], f32, tag="bh")
    nc.sync.dma_start(bh_sb[:], bh_d[:, :])

    ctx_all = ctx_pool.tile([P, NH * BL], f32, tag="ctxall")

    # fp32 enc tiles stream on the gpsimd queue (sync queue carries bf16+Wh)
    def load_ef(b):
        tiles = []
        for h in range(NH):
            t_ = ef_pool.tile([P, T], f32, tag=f"ef{h}")
            nc.sync.dma_start(t_[:], enc_f32_d[h * P:(h + 1) * P,
                                                 b * T:(b + 1) * T])
            tiles.append(t_)
        return tiles

    ef_cur = load_ef(0)

    state = {}  # batch -> (tanh tiles, ef tiles)

    def issue_score_and_context(b, tanh_ts, ef, last):
        """Score matmuls + softmax + context for batch b."""
        ps = sc_pool.tile([1, T], f32, tag="score")
        for j in range(NU):
            nc.tensor.matmul(
                ps[:], wv_sb[:, j:j + 1], tanh_ts[j][:],
                start=(j == 0), stop=(j == NU - 1), skip_group_check=True,
            )
        nmax = row_pool.tile([1, 1], f32, tag="nmax")
        nc.vector.tensor_reduce(nmax[:], ps[:], axis=mybir.AxisListType.X,
                                op=ALU.max, negate=True)
        erow = row_pool.tile([1, T], f32, tag="erow")
        ssum = row_pool.tile([1, 1], f32, tag="ssum")
        nc.scalar.activation(erow[:], ps[:], AF.Exp, bias=nmax[:],
                             accum_out=ssum[:])
        rrec = row_pool.tile([1, 1], f32, tag="rrec")
        nc.vector.reciprocal(rrec[:], ssum[:])
        arow = row_pool.tile([1, T], f32, tag="arow")
        nc.vector.tensor_scalar_mul(arow[:], erow[:], rrec[:])
        nc.sync.dma_start(attn_d[b:b + 1, :], arow[:])

        bc = bc_pool.tile([P, T], f32, tag="bc")
        nc.gpsimd.partition_broadcast(bc[:], arow[:])
        for h in range(NH):
            scr = scr_pool.tile([P, T], f32, tag="scr")
            nc.vector.tensor_mul(scr[:], ef[h][:], bc[:])
            col = ctx_all[:, h * BL + b:h * BL + b + 1]
            if last:
                # keep the exposed tail short: reduce on ScalarE via the
                # activation accumulator while VectorE keeps multiplying
                scr2 = scr_pool.tile([P, T], f32, tag="scr2")
                nc.scalar.activation(scr2[:], scr[:], AF.Identity, bias=0.0,
                                     accum_out=col)
            else:
                nc.vector.tensor_reduce(col, scr[:],
                                        axis=mybir.AxisListType.X, op=ALU.add)

    for b in range(BL):
        ebf, ef = ebf_cur, ef_cur
        tanh_ts = []
        for j in range(NU):
            pe = pe_pool.tile([P, T], f32, tag="pe")
            for h in range(NH):
                nc.tensor.matmul(
                    pe[:], wh_t[j][:, h * P:(h + 1) * P], ebf[h][:],
                    start=(h == 0), stop=(h == NH - 1),
                )
            th = th_pool.tile([P, T], bf16, tag=f"th{j}")
            nc.scalar.activation(th[:], pe[:], AF.Tanh, bias=bh_sb[:, j:j + 1])
            tanh_ts.append(th)
            if j == 0 and b > 0:
                # previous batch's score matmuls slot in here: their tanh
                # inputs are long since ready, so the PE stays dense
                pt, pef = state.pop(b - 1)
                issue_score_and_context(b - 1, pt, pef, last=False)
            if j == 1 and b + 1 < BL:
                # prefetch next batch while this one computes
                nxt = []
                for h in range(NH):
                    t_ = ebf_pool.tile([P, T], bf16, tag=f"ebf{h}")
                    nc.sync.dma_start(
                        t_[:], enc_bf_d[h * P:(h + 1) * P,
                                        (b + 1) * T:(b + 2) * T])
                    nxt.append(t_)
                ebf_cur = nxt
                ef_cur = load_ef(b + 1)
        state[b] = (tanh_ts, ef)

    pt, pef = state.pop(BL - 1)
    issue_score_and_context(BL - 1, pt, pef, last=True)

    nc.sync.dma_start(ctxt_d[:, :], ctx_all[:])


def build_nc():
    nc = bacc.Bacc("TRN2", target_bir_lowering=False, debug=False,
                   num_devices=NCORES)
    aps = {
        "enc_bf": nc.dram_tensor("enc_bf", [H, BL * T], bf16,
                                 kind="ExternalInput").ap(),
        "enc_f32": nc.dram_tensor("enc_f32", [H, BL * T], f32,
                                  kind="ExternalInput").ap(),
        "wh": nc.dram_tensor("wh", [NU, H, P], bf16,
                             kind="ExternalInput").ap(),
        "wv": nc.dram_tensor("wv", [P, NU], bf16, kind="ExternalInput").ap(),
        "bh": nc.dram_tensor("bh", [P, NU], f32, kind="ExternalInput").ap(),
        "attn": nc.dram_tensor("attn", [BL, T], f32,
                               kind="ExternalOutput").ap(),
        "ctxt": nc.dram_tensor("ctxt", [P, NH * BL], f32,
                               kind="ExternalOutput").ap(),
    }
    with tile.TileContext(nc) as tc:
        with ExitStack() as body_ctx:
            build_kernel_body(tc, aps, body_ctx)
    nc.compile()
    return nc


def make_in_maps(enc_output, Wh, bh, Wv):
    enc = np.ascontiguousarray(np.asarray(enc_output, dtype=np.float32))
    wh = np.asarray(Wh, dtype=np.float32)
    # j-tiled Wh: wh_tiled[j, h*P+p, c] = Wh[h*P+p, j*P+c]
    wh_tiled = np.ascontiguousarray(
        wh.reshape(H, NU, P).transpose(1, 0, 2)).astype(BF16)
    wv_t = np.ascontiguousarray(
        np.asarray(Wv, dtype=np.float32).reshape(NU, P).T).astype(BF16)
    bh_t = np.ascontiguousarray(
        np.asarray(bh, dtype=np.float32).reshape(NU, P).T)
    in_maps = []
    for c in range(NCORES):
        shard = enc[c * BL:(c + 1) * BL].reshape(BL * T, H)
        encT = np.ascontiguousarray(shard.T)          # [H, BL*T] f32
        in_maps.append({
            "enc_bf": encT.astype(BF16),
            "enc_f32": encT,
            "wh": wh_tiled,
            "wv": wv_t,
            "bh": bh_t,
        })
    return in_maps


_NC_CACHE = None


def kernel(dec_hidden, enc_output, Wh, bh, Ws, bs, Wv, bv, **_unused):
    global _NC_CACHE
    if _NC_CACHE is None:
        _NC_CACHE = build_nc()
    nc = _NC_CACHE
    in_maps = make_in_maps(enc_output, Wh, bh, Wv)
    res = run_bass_kernel_spmd(nc, in_maps, list(range(NCORES))).results
    attn = np.concatenate([res[c]["attn"] for c in range(NCORES)], axis=0)
    ctx_parts = []
    for c in range(NCORES):
        ct = res[c]["ctxt"]                           # [P, NH*BL]
        ct = ct.reshape(P, NH, BL).transpose(2, 1, 0).reshape(BL, H)
        ctx_parts.append(np.ascontiguousarray(ct))
    context = np.concatenate(ctx_parts, axis=0)
    return context.astype(np.float32), attn.astype(np.float32)
